# revision 32
# baseline (speedup 1.0000x reference)
"""Average Hausdorff loss on 8 Trainium2 NeuronCores.

Strategy (v4: one-matrix shared blocks, pruned, host dp-finish)
--------------------------------------------------------------
Host (numpy, cheap):
  * binarize + 3x3-erosion edge detection, compact edge coords per (b,c)
  * per pair: KD-split the GTH points into tiles of <=128. A cell-grid
    separable EDT gives per-point NN-distance upper bounds in both
    directions. A tile's candidate set = pred points p with
    dist(p, sub-bbox) <= max(UB_sub, UB_p), which provably contains
      - the NN pred point of every gth query in the tile, and
      - every pred point whose own NN gth point lies in the tile.
    So ONE distance block per gth tile serves both directions.
  * blocks are cut into uniform 128-candidate pieces, bin-packed over
    the 8 cores into uniform job slots (device program is piece-index
    uniform; per-core variation is data only).

Device (raw Bass, SPMD over 8 cores):
  PE  : merged matmuls [6,128]^T @ [6,<=512] -> PSUM -(d^2)/4 (exact
        bf16 via byte-split squared norms), 4 ping-pong PSUM regions.
  ACT : copies each 8-piece group to an SBUF fp16 ring (scale 2^-12).
  DVE : fp16 tensor_max fold + [128,8,64] tensor_reduce -> per-gth-
        query NN column per piece (g->p direction).
  DMA : streams every drained fp16 block back to DRAM.
Host: g->p from the NN columns; p->g by per-column max over the 128
partitions of the returned blocks, scatter-min by candidate id; sqrt,
means, nanmean.
"""

import math
import numpy as np

H = 256
W_IMG = 256
BC = 16
N_CORES = 8
TILE_Q = 128
SUB_Q = 8
CELL = 2
WP = 128              # uniform piece width (candidate cols)
GT = 8                # pieces per reduce group (1024 cols)
SENT = 16384.0
D2_SCALE = 2.0 ** -12
D2_BACK = -4.0 * 4096.0


def _edge_maps(x):
    m = x > 0.5
    p = np.pad(m, ((0, 0), (1, 1), (1, 1)), constant_values=True)
    e = np.ones_like(m)
    for dy in range(3):
        for dx in range(3):
            e &= p[:, dy:dy + H, dx:dx + W_IMG]
    return m & ~e


def _aug_g(cy, cx, n_pad):
    n = cy.shape[0]
    fy = np.full(n_pad, SENT, np.float32)
    fx = np.full(n_pad, SENT, np.float32)
    fy[:n] = cy
    fx[:n] = cx
    sq = fy * fy + fx * fx
    b1 = np.floor(sq / 256.0)
    b0 = sq - b1 * 256.0
    out = np.empty((6, n_pad), np.float32)
    out[0] = fy * 0.5
    out[1] = fx * 0.5
    out[2] = -b1
    out[3] = -b0
    out[4] = -64.0
    out[5] = -0.25
    return out


def _aug_p(cy, cx, n_pad):
    n = cy.shape[0]
    fy = np.full(n_pad, SENT, np.float32)
    fx = np.full(n_pad, SENT, np.float32)
    fy[:n] = cy
    fx[:n] = cx
    sq = fy * fy + fx * fx
    b1 = np.floor(sq / 256.0)
    b0 = sq - b1 * 256.0
    out = np.empty((6, n_pad), np.float32)
    out[0] = fy
    out[1] = fx
    out[2] = 64.0
    out[3] = 0.25
    out[4] = b1
    out[5] = b0
    return out


def _kd_tiles(ys, xs, tile):
    out = []

    def rec(ix):
        if len(ix) <= tile:
            out.append(ix)
            return
        yy, xx = ys[ix], xs[ix]
        k = yy if (yy.max() - yy.min() >= xx.max() - xx.min()) else xx
        n = len(ix)
        half = (n // 2 // tile) * tile or n // 2
        o = np.argsort(k, kind="stable")
        rec(ix[o[:half]])
        rec(ix[o[half:]])

    rec(np.arange(len(ys)))
    return out


def _cell_ub(dys, dxs):
    G = 256 // CELL
    occ = np.zeros((G, G), bool)
    occ[dys // CELL, dxs // CELL] = True
    BIG = np.int64(10 ** 9)
    ar = np.arange(G)
    d2 = (ar[:, None] - ar[None, :]) ** 2
    occf = np.where(occ, 0, BIG)
    gcol = (d2[:, :, None] + occf[None, :, :]).min(axis=1)
    D2 = (gcol[:, None, :] + d2[None, :, :]).min(axis=2)
    return np.sqrt(D2.astype(np.float64)) * CELL + math.sqrt(2.0) * CELL


def _build_jobs_shared(gy, gx, py, px):
    """Per pair: one job per gth tile; candidates serve both directions."""
    ub_g2p = _cell_ub(py, px)[gy // CELL, gx // CELL]
    ub_p2g = _cell_ub(gy, gx)[py // CELL, px // CELL]
    jobs = []
    for ix in _kd_tiles(gy, gx, TILE_Q):
        m = np.zeros(len(py), bool)
        for s in range(0, len(ix), SUB_Q):
            sx = ix[s:s + SUB_Q]
            u = ub_g2p[sx].max()
            y0, y1 = gy[sx].min(), gy[sx].max()
            x0, x1 = gx[sx].min(), gx[sx].max()
            dy = np.maximum(np.maximum(y0 - py, py - y1), 0)
            dx = np.maximum(np.maximum(x0 - px, px - x1), 0)
            dd = np.sqrt((dy * dy + dx * dx).astype(np.float64))
            m |= (dd <= u) | (dd <= ub_p2g)
        jobs.append((ix, np.nonzero(m)[0]))
    return jobs


def _build_program(n_pieces, piece_job, mov_chunks_sync, mov_chunks_gps,
                   stat_split):
    from contextlib import ExitStack
    import concourse.bass as bass
    import concourse.mybir as mybir

    f32 = mybir.dt.float32
    f16 = mybir.dt.float16
    bf16 = mybir.dt.bfloat16
    MAX = mybir.AluOpType.max
    X = mybir.AxisListType.X

    groups = []
    p = 0
    while p < n_pieces:
        # smaller groups near the end shorten the pipeline-drain tail
        t = GT if n_pieces - p >= 3 * GT else min(GT // 2, n_pieces - p)
        groups.append((p, p + t))
        p += t
    n_grp = len(groups)

    nc = bass.Bass()
    stat_d = nc.declare_dram_parameter("stat", [6, 128 * n_pieces], bf16,
                                       isOutput=False)
    mov_d = nc.declare_dram_parameter("mov", [6, WP * n_pieces], bf16,
                                      isOutput=False)
    tmpa_d = nc.declare_dram_parameter("tmpa", [128, n_pieces], f16,
                                       isOutput=True)
    blk_d = nc.declare_dram_parameter("blk", [128, WP * n_pieces], f16,
                                      isOutput=True)

    def group_matmuls(lo, hi):
        mms = []
        i = lo
        while i < hi:
            j = i
            while (j + 1 < hi and piece_job[j + 1] == piece_job[i]
                   and (j + 1 - lo) % 4 != 0):
                j += 1
            mms.append((i, j - i + 1))
            i = j + 1
        return mms

    with ExitStack() as ctx:
        stat = ctx.enter_context(
            nc.sbuf_tensor("stat_s", [6, 128 * n_pieces], bf16))
        mov = ctx.enter_context(
            nc.sbuf_tensor("mov_s", [6, WP * n_pieces], bf16))
        tmpa = ctx.enter_context(
            nc.sbuf_tensor("tmpa_s", [128, n_pieces], f16))
        act_ring = [ctx.enter_context(
            nc.sbuf_tensor(f"actr{i}", [128, GT, WP], f16)) for i in range(4)]
        fold = ctx.enter_context(
            nc.sbuf_tensor("fold_s", [128, GT, WP // 2], f16))
        psum = ctx.enter_context(
            nc.psum_tensor("ps", [128, 4096 // WP, WP], f32))

        stat_sem = ctx.enter_context(nc.semaphore("stat_in"))
        mov_sem = ctx.enter_context(nc.semaphore("mov_in"))
        mov2_sem = ctx.enter_context(nc.semaphore("mov2_in"))
        pe_sem = ctx.enter_context(nc.semaphore("pe_done"))
        act_sem = ctx.enter_context(nc.semaphore("act_done"))
        ta_sem = ctx.enter_context(nc.semaphore("tailA"))
        blk_sem = ctx.enter_context(nc.semaphore("blk_out"))
        out_sem = ctx.enter_context(nc.semaphore("dma_out"))
        block = ctx.enter_context(nc.Block(no_gpsimd_drain=True))

        sync_need = np.zeros(n_pieces, np.int64)
        for c, (p0, p1) in enumerate(mov_chunks_sync + mov_chunks_gps):
            sync_need[p0:p1] = c + 1
        sync_need = np.maximum.accumulate(sync_need)

        @block.sync
        def _(sync):
            s1 = stat_split * 128
            sync.dma_start(stat[:, 0:s1], stat_d[:, 0:s1]).then_inc(stat_sem, 16)
            first = True
            for (p0, p1) in mov_chunks_sync + mov_chunks_gps:
                sync.dma_start(mov[:, p0 * WP:p1 * WP],
                               mov_d[:, p0 * WP:p1 * WP]).then_inc(mov_sem, 16)
                if first:
                    sync.dma_start(stat[:, s1:], stat_d[:, s1:]).then_inc(
                        stat_sem, 16)
                    first = False
            # stream drained blocks out as they are produced
            for k, (lo, hi) in enumerate(groups):
                sync.wait_ge(act_sem, k + 1)
                t = hi - lo
                sync.dma_start(
                    blk_d[:, lo * WP:hi * WP],
                    act_ring[k % 4][:, 0:t, :].rearrange("p a b -> p (a b)"),
                ).then_inc(blk_sem, 16)
            sync.wait_ge(ta_sem, n_grp)
            sync.dma_start(tmpa_d[:], tmpa[:]).then_inc(out_sem, 16)



        @block.tensor
        def _(tensor):
            tensor.wait_ge(stat_sem, 16)
            s_seen = 0
            st_seen = 1
            for k, (lo, hi) in enumerate(groups):
                if k >= 3:
                    tensor.wait_ge(act_sem, k - 2)
                if hi - 1 >= stat_split and st_seen < 2:
                    tensor.wait_ge(stat_sem, 32)
                    st_seen = 2
                need_s = int(sync_need[hi - 1])
                if need_s > s_seen:
                    tensor.wait_ge(mov_sem, 16 * need_s)
                    s_seen = need_s
                base = (k % 4) * GT
                mms = group_matmuls(lo, hi)
                for mi, (plo, np_) in enumerate(mms):
                    slot = base + (plo - lo)
                    mm = nc.tensor.matmul(
                        psum[:].rearrange("p a b -> p (a b)")
                            [:, slot * WP:(slot + np_) * WP],
                        stat[:, plo * 128:(plo + 1) * 128],
                        mov[:, plo * WP:(plo + np_) * WP],
                        start=True, stop=True,
                    )
                    if mi == len(mms) - 1:
                        mm.then_inc(pe_sem, 1)

        @block.scalar
        def _(scalar):
            for k, (lo, hi) in enumerate(groups):
                scalar.wait_ge(pe_sem, k + 1)
                if k >= 3:
                    scalar.wait_ge(ta_sem, k - 2)
                    scalar.wait_ge(blk_sem, 16 * (k - 2))
                t = hi - lo
                base = (k % 4) * GT
                src = psum[:, base:base + t, :]
                dst = act_ring[k % 4][:, 0:t, :]
                nc.scalar.activation(
                    dst.rearrange("p a b -> p (a b)"),
                    src.rearrange("p a b -> p (a b)"),
                    mybir.ActivationFunctionType.Copy, scale=D2_SCALE,
                ).then_inc(act_sem, 1)

        @block.vector
        def _(vector):
            for k, (lo, hi) in enumerate(groups):
                t = hi - lo
                vector.wait_ge(act_sem, k + 1)
                buf = act_ring[k % 4]
                nc.vector.tensor_max(
                    fold[:, 0:t, :],
                    buf[:, 0:t, 0:WP // 2], buf[:, 0:t, WP // 2:WP],
                )
                nc.vector.tensor_reduce(
                    tmpa[:, lo:hi], fold[:, 0:t, :],
                    axis=X, op=MAX,
                ).then_inc(ta_sem, 1)

    return nc


def _loss_from_sums(sg, ng, sp, npnts):
    with np.errstate(divide="ignore", invalid="ignore"):
        g2p = sg / ng if ng > 0 else np.float64(np.nan)
        p2g = sp / npnts if npnts > 0 else np.float64(np.nan)
        if ng == 0 and npnts == 0:
            return np.float64(np.nan)
        ahd = (g2p + p2g) / 2.0
        return 1.0 - 1.0 / (1.0 + ahd)


RUN_OPTS = {}
LAST_RES = None
LAST_NN = None


def kernel(gth, pred):
    from concourse.bass_utils import run_bass_kernel_spmd
    import ml_dtypes

    gth = np.asarray(gth, np.float32).reshape(BC, H, W_IMG)
    pred = np.asarray(pred, np.float32).reshape(BC, H, W_IMG)

    gedge = _edge_maps(gth)
    pedge = _edge_maps(pred)

    all_jobs = []      # (npc, pair, q_ix, c_ix)
    pts = []
    for i in range(BC):
        gy, gx = np.nonzero(gedge[i])
        py, px = np.nonzero(pedge[i])
        pts.append((gy.astype(np.float32) - 128.0, gx.astype(np.float32) - 128.0,
                    py.astype(np.float32) - 128.0, px.astype(np.float32) - 128.0))
        if len(gy) and len(py):
            for q_ix, c_ix in _build_jobs_shared(gy, gx, py, px):
                npc = max(1, -(-len(c_ix) // WP))
                all_jobs.append((npc, i, q_ix, c_ix))

    order = sorted(range(len(all_jobs)),
                   key=lambda k: all_jobs[k][0], reverse=True)
    loads = [0] * N_CORES
    per_core = [[] for _ in range(N_CORES)]
    for k in order:
        c = min(range(N_CORES), key=lambda q: loads[q])
        per_core[c].append(k)
        loads[c] += all_jobs[k][0]

    for c in range(N_CORES):
        per_core[c].sort(key=lambda k: all_jobs[k][0], reverse=True)
    J = max(len(v) for v in per_core)
    slot_w = [0] * J
    for c in range(N_CORES):
        for j, k in enumerate(per_core[c]):
            slot_w[j] = max(slot_w[j], all_jobs[k][0])
    P = sum(slot_w)
    piece_job = np.zeros(P, np.int64)
    slot_off = []
    p = 0
    for j, w in enumerate(slot_w):
        slot_off.append(p)
        piece_job[p:p + w] = j
        p += w

    c0 = min(3 * GT, P)
    stat_split = c0
    rest = P - c0
    bnds = [0, c0] + [c0 + rest * t // 4 for t in (1, 2, 3)] + [P]
    bnds = sorted(set(bnds))
    mov_chunks_sync = [(bnds[t], bnds[t + 1]) for t in range(len(bnds) - 1)]
    mov_chunks_gps = []

    nc = _build_program(P, piece_job, mov_chunks_sync, mov_chunks_gps,
                        stat_split)

    sent_stat = _aug_g(np.empty(0, np.float32), np.empty(0, np.float32), 128)
    sent_mov = _aug_p(np.empty(0, np.float32), np.empty(0, np.float32), WP)
    in_maps = []
    piece_map = []
    for c in range(N_CORES):
        stat = np.empty((6, 128 * P), np.float32)
        mov = np.empty((6, WP * P), np.float32)
        pmap = [None] * P
        for j in range(J):
            p = slot_off[j]
            w = slot_w[j]
            if j < len(per_core[c]):
                k = per_core[c][j]
                npc, i, q_ix, c_ix = all_jobs[k]
                gy, gx, py, px = pts[i]
                sa = _aug_g(gy[q_ix], gx[q_ix], 128)
                aug = _aug_p(py[c_ix], px[c_ix], w * WP)
                for t in range(w):
                    stat[:, (p + t) * 128:(p + t + 1) * 128] = sa
                    mov[:, (p + t) * WP:(p + t + 1) * WP] = \
                        aug[:, t * WP:(t + 1) * WP]
                    pmap[p + t] = k
            else:
                for t in range(w):
                    stat[:, (p + t) * 128:(p + t + 1) * 128] = sent_stat
                    mov[:, (p + t) * WP:(p + t + 1) * WP] = sent_mov
        piece_map.append(pmap)
        in_maps.append({
            "stat": stat.astype(ml_dtypes.bfloat16),
            "mov": mov.astype(ml_dtypes.bfloat16),
        })

    res = run_bass_kernel_spmd(nc, in_maps, list(range(N_CORES)), **RUN_OPTS)
    global LAST_RES, LAST_NN
    LAST_RES = res

    sums = np.zeros((BC, 2), np.float64)
    nn_dbg = {}
    # p->g per-pair NN accumulator over all pred points
    dp_min = [np.full(len(pts[i][2]), np.inf) for i in range(BC)]
    for c in range(N_CORES):
        tmpav = np.asarray(res.results[c]["tmpa"], np.float64)   # [128, P]
        blk = np.asarray(res.results[c]["blk"], np.float64)      # [128, P*WP]
        # group pieces by job slot
        seen = set()
        for pi, k in enumerate(piece_map[c]):
            if k is None or k in seen:
                continue
            seen.add(k)
            npc, i, q_ix, c_ix = all_jobs[k]
            j = per_core[c].index(k)
            p0 = slot_off[j]
            w = slot_w[j]
            nq = len(q_ix)
            # g->p: min over the job's piece columns
            d2 = (tmpav[:nq, p0:p0 + w] * D2_BACK).min(axis=1)
            dist = np.sqrt(np.maximum(d2, 0.0))
            sums[i, 0] += dist.sum()
            nn_dbg.setdefault((i, 0), []).append((q_ix, dist))
            # p->g: per-column max over partitions, scatter-min
            v = blk[:, p0 * WP:p0 * WP + len(c_ix)].max(axis=0)
            d2p = v * D2_BACK
            np.minimum.at(dp_min[i], c_ix, d2p)
    LAST_NN = nn_dbg

    losses = np.full(BC, np.nan, np.float64)
    for i in range(BC):
        gy = pts[i][0]
        py = pts[i][2]
        n_g, n_p = len(gy), len(py)
        if n_g == 0 and n_p == 0:
            continue
        if n_g == 0 or n_p == 0:
            losses[i] = _loss_from_sums(np.inf, max(n_g, 1),
                                        np.inf, max(n_p, 1))
        else:
            sp = np.sqrt(np.maximum(dp_min[i], 0.0)).sum()
            losses[i] = _loss_from_sums(sums[i, 0], n_g, sp, n_p)

    return np.float32(np.nanmean(losses.astype(np.float32)))


# revision 33
# speedup vs baseline: 1.0157x; 1.0157x over previous
"""Average Hausdorff loss on 8 Trainium2 NeuronCores.

Strategy (v4: one-matrix shared blocks, pruned, host dp-finish)
--------------------------------------------------------------
Host (numpy, cheap):
  * binarize + 3x3-erosion edge detection, compact edge coords per (b,c)
  * per pair: KD-split the GTH points into tiles of <=128. A cell-grid
    separable EDT gives per-point NN-distance upper bounds in both
    directions. A tile's candidate set = pred points p with
    dist(p, sub-bbox) <= max(UB_sub, UB_p), which provably contains
      - the NN pred point of every gth query in the tile, and
      - every pred point whose own NN gth point lies in the tile.
    So ONE distance block per gth tile serves both directions.
  * blocks are cut into uniform 128-candidate pieces, bin-packed over
    the 8 cores into uniform job slots (device program is piece-index
    uniform; per-core variation is data only).

Device (raw Bass, SPMD over 8 cores):
  PE  : merged matmuls [6,128]^T @ [6,<=512] -> PSUM -(d^2)/4 (exact
        bf16 via byte-split squared norms), 4 ping-pong PSUM regions.
  ACT : copies each 8-piece group to an SBUF fp16 ring (scale 2^-12).
  DVE : fp16 tensor_max fold + [128,8,64] tensor_reduce -> per-gth-
        query NN column per piece (g->p direction).
  DMA : streams every drained fp16 block back to DRAM.
Host: g->p from the NN columns; p->g by per-column max over the 128
partitions of the returned blocks, scatter-min by candidate id; sqrt,
means, nanmean.
"""

import math
import numpy as np

H = 256
W_IMG = 256
BC = 16
N_CORES = 8
TILE_Q = 128
SUB_Q = 8
CELL = 2
WP = 128              # uniform piece width (candidate cols)
GT = 8                # pieces per reduce group (1024 cols)
SENT = 16384.0
D2_SCALE = 2.0 ** -12
D2_BACK = -4.0 * 4096.0


def _edge_maps(x):
    m = x > 0.5
    p = np.pad(m, ((0, 0), (1, 1), (1, 1)), constant_values=True)
    e = np.ones_like(m)
    for dy in range(3):
        for dx in range(3):
            e &= p[:, dy:dy + H, dx:dx + W_IMG]
    return m & ~e


def _aug_g(cy, cx, n_pad):
    n = cy.shape[0]
    fy = np.full(n_pad, SENT, np.float32)
    fx = np.full(n_pad, SENT, np.float32)
    fy[:n] = cy
    fx[:n] = cx
    sq = fy * fy + fx * fx
    b1 = np.floor(sq / 256.0)
    b0 = sq - b1 * 256.0
    out = np.empty((6, n_pad), np.float32)
    out[0] = fy * 0.5
    out[1] = fx * 0.5
    out[2] = -b1
    out[3] = -b0
    out[4] = -64.0
    out[5] = -0.25
    return out


def _aug_p(cy, cx, n_pad):
    n = cy.shape[0]
    fy = np.full(n_pad, SENT, np.float32)
    fx = np.full(n_pad, SENT, np.float32)
    fy[:n] = cy
    fx[:n] = cx
    sq = fy * fy + fx * fx
    b1 = np.floor(sq / 256.0)
    b0 = sq - b1 * 256.0
    out = np.empty((6, n_pad), np.float32)
    out[0] = fy
    out[1] = fx
    out[2] = 64.0
    out[3] = 0.25
    out[4] = b1
    out[5] = b0
    return out


def _kd_tiles(ys, xs, tile):
    out = []

    def rec(ix):
        if len(ix) <= tile:
            out.append(ix)
            return
        yy, xx = ys[ix], xs[ix]
        k = yy if (yy.max() - yy.min() >= xx.max() - xx.min()) else xx
        n = len(ix)
        half = (n // 2 // tile) * tile or n // 2
        o = np.argsort(k, kind="stable")
        rec(ix[o[:half]])
        rec(ix[o[half:]])

    rec(np.arange(len(ys)))
    return out


def _cell_ub(dys, dxs):
    G = 256 // CELL
    occ = np.zeros((G, G), bool)
    occ[dys // CELL, dxs // CELL] = True
    BIG = np.int64(10 ** 9)
    ar = np.arange(G)
    d2 = (ar[:, None] - ar[None, :]) ** 2
    occf = np.where(occ, 0, BIG)
    gcol = (d2[:, :, None] + occf[None, :, :]).min(axis=1)
    D2 = (gcol[:, None, :] + d2[None, :, :]).min(axis=2)
    return np.sqrt(D2.astype(np.float64)) * CELL + math.sqrt(2.0) * CELL


def _build_jobs_shared(gy, gx, py, px):
    """Per pair: one job per gth tile; candidates serve both directions."""
    ub_g2p = _cell_ub(py, px)[gy // CELL, gx // CELL]
    ub_p2g = _cell_ub(gy, gx)[py // CELL, px // CELL]
    jobs = []
    for ix in _kd_tiles(gy, gx, TILE_Q):
        m = np.zeros(len(py), bool)
        for s in range(0, len(ix), SUB_Q):
            sx = ix[s:s + SUB_Q]
            u = ub_g2p[sx].max()
            y0, y1 = gy[sx].min(), gy[sx].max()
            x0, x1 = gx[sx].min(), gx[sx].max()
            dy = np.maximum(np.maximum(y0 - py, py - y1), 0)
            dx = np.maximum(np.maximum(x0 - px, px - x1), 0)
            dd = np.sqrt((dy * dy + dx * dx).astype(np.float64))
            m |= (dd <= u) | (dd <= ub_p2g)
        jobs.append((ix, np.nonzero(m)[0]))
    return jobs


def _build_program(n_pieces, piece_job, mov_chunks_sync, mov_chunks_gps,
                   stat_split):
    from contextlib import ExitStack
    import concourse.bass as bass
    import concourse.mybir as mybir

    f32 = mybir.dt.float32
    f16 = mybir.dt.float16
    bf16 = mybir.dt.bfloat16
    MAX = mybir.AluOpType.max
    X = mybir.AxisListType.X

    groups = []
    p = 0
    while p < n_pieces:
        # smaller groups near the end shorten the pipeline-drain tail
        t = GT if n_pieces - p >= 3 * GT else min(GT // 2, n_pieces - p)
        groups.append((p, p + t))
        p += t
    n_grp = len(groups)

    nc = bass.Bass()
    stat_d = nc.declare_dram_parameter("stat", [6, 128 * n_pieces], bf16,
                                       isOutput=False)
    mov_d = nc.declare_dram_parameter("mov", [6, WP * n_pieces], bf16,
                                      isOutput=False)
    tmpa_d = nc.declare_dram_parameter("tmpa", [128, n_pieces], f16,
                                       isOutput=True)
    blk_d = nc.declare_dram_parameter("blk", [128, WP * n_pieces], f16,
                                      isOutput=True)

    def group_matmuls(lo, hi):
        mms = []
        i = lo
        while i < hi:
            j = i
            while (j + 1 < hi and piece_job[j + 1] == piece_job[i]
                   and (j + 1 - lo) % 4 != 0):
                j += 1
            mms.append((i, j - i + 1))
            i = j + 1
        return mms

    with ExitStack() as ctx:
        stat = ctx.enter_context(
            nc.sbuf_tensor("stat_s", [6, 128 * n_pieces], bf16))
        mov = ctx.enter_context(
            nc.sbuf_tensor("mov_s", [6, WP * n_pieces], bf16))
        tmpa = ctx.enter_context(
            nc.sbuf_tensor("tmpa_s", [128, n_pieces], f16))
        act_ring = [ctx.enter_context(
            nc.sbuf_tensor(f"actr{i}", [128, GT, WP], f16)) for i in range(4)]
        fold = ctx.enter_context(
            nc.sbuf_tensor("fold_s", [128, GT, WP // 2], f16))
        psum = ctx.enter_context(
            nc.psum_tensor("ps", [128, 4096 // WP, WP], f32))

        stat_sem = ctx.enter_context(nc.semaphore("stat_in"))
        mov_sem = ctx.enter_context(nc.semaphore("mov_in"))
        mov2_sem = ctx.enter_context(nc.semaphore("mov2_in"))
        pe_sem = ctx.enter_context(nc.semaphore("pe_done"))
        act_sem = ctx.enter_context(nc.semaphore("act_done"))
        ta_sem = ctx.enter_context(nc.semaphore("tailA"))
        blk_sem = ctx.enter_context(nc.semaphore("blk_out"))
        out_sem = ctx.enter_context(nc.semaphore("dma_out"))
        block = ctx.enter_context(nc.Block())

        sync_need = np.zeros(n_pieces, np.int64)
        for c, (p0, p1) in enumerate(mov_chunks_sync + mov_chunks_gps):
            sync_need[p0:p1] = c + 1
        sync_need = np.maximum.accumulate(sync_need)

        @block.sync
        def _(sync):
            s1 = stat_split * 128
            sync.dma_start(stat[:, 0:s1], stat_d[:, 0:s1]).then_inc(stat_sem, 16)
            first = True
            for (p0, p1) in mov_chunks_sync + mov_chunks_gps:
                sync.dma_start(mov[:, p0 * WP:p1 * WP],
                               mov_d[:, p0 * WP:p1 * WP]).then_inc(mov_sem, 16)
                if first:
                    sync.dma_start(stat[:, s1:], stat_d[:, s1:]).then_inc(
                        stat_sem, 16)
                    first = False
            # stream drained blocks out as they are produced
            for k, (lo, hi) in enumerate(groups):
                sync.wait_ge(act_sem, k + 1)
                t = hi - lo
                sync.dma_start(
                    blk_d[:, lo * WP:hi * WP],
                    act_ring[k % 4][:, 0:t, :].rearrange("p a b -> p (a b)"),
                ).then_inc(blk_sem, 16)
            sync.wait_ge(ta_sem, n_grp)
            sync.dma_start(tmpa_d[:], tmpa[:]).then_inc(out_sem, 16)



        @block.tensor
        def _(tensor):
            tensor.wait_ge(stat_sem, 16)
            s_seen = 0
            st_seen = 1
            for k, (lo, hi) in enumerate(groups):
                if k >= 3:
                    tensor.wait_ge(act_sem, k - 2)
                if hi - 1 >= stat_split and st_seen < 2:
                    tensor.wait_ge(stat_sem, 32)
                    st_seen = 2
                need_s = int(sync_need[hi - 1])
                if need_s > s_seen:
                    tensor.wait_ge(mov_sem, 16 * need_s)
                    s_seen = need_s
                base = (k % 4) * GT
                mms = group_matmuls(lo, hi)
                for mi, (plo, np_) in enumerate(mms):
                    slot = base + (plo - lo)
                    mm = nc.tensor.matmul(
                        psum[:].rearrange("p a b -> p (a b)")
                            [:, slot * WP:(slot + np_) * WP],
                        stat[:, plo * 128:(plo + 1) * 128],
                        mov[:, plo * WP:(plo + np_) * WP],
                        start=True, stop=True,
                    )
                    if mi == len(mms) - 1:
                        mm.then_inc(pe_sem, 1)

        @block.scalar
        def _(scalar):
            for k, (lo, hi) in enumerate(groups):
                scalar.wait_ge(pe_sem, k + 1)
                if k >= 3:
                    scalar.wait_ge(ta_sem, k - 2)
                    scalar.wait_ge(blk_sem, 16 * (k - 2))
                t = hi - lo
                base = (k % 4) * GT
                src = psum[:, base:base + t, :]
                dst = act_ring[k % 4][:, 0:t, :]
                nc.scalar.activation(
                    dst.rearrange("p a b -> p (a b)"),
                    src.rearrange("p a b -> p (a b)"),
                    mybir.ActivationFunctionType.Copy, scale=D2_SCALE,
                ).then_inc(act_sem, 1)

        @block.vector
        def _(vector):
            for k, (lo, hi) in enumerate(groups):
                t = hi - lo
                vector.wait_ge(act_sem, k + 1)
                buf = act_ring[k % 4]
                nc.vector.tensor_max(
                    fold[:, 0:t, :],
                    buf[:, 0:t, 0:WP // 2], buf[:, 0:t, WP // 2:WP],
                )
                nc.vector.tensor_reduce(
                    tmpa[:, lo:hi], fold[:, 0:t, :],
                    axis=X, op=MAX,
                ).then_inc(ta_sem, 1)

    return nc


def _loss_from_sums(sg, ng, sp, npnts):
    with np.errstate(divide="ignore", invalid="ignore"):
        g2p = sg / ng if ng > 0 else np.float64(np.nan)
        p2g = sp / npnts if npnts > 0 else np.float64(np.nan)
        if ng == 0 and npnts == 0:
            return np.float64(np.nan)
        ahd = (g2p + p2g) / 2.0
        return 1.0 - 1.0 / (1.0 + ahd)


RUN_OPTS = {}
LAST_RES = None
LAST_NN = None


def kernel(gth, pred):
    from concourse.bass_utils import run_bass_kernel_spmd
    import ml_dtypes

    gth = np.asarray(gth, np.float32).reshape(BC, H, W_IMG)
    pred = np.asarray(pred, np.float32).reshape(BC, H, W_IMG)

    gedge = _edge_maps(gth)
    pedge = _edge_maps(pred)

    all_jobs = []      # (npc, pair, q_ix, c_ix)
    pts = []
    for i in range(BC):
        gy, gx = np.nonzero(gedge[i])
        py, px = np.nonzero(pedge[i])
        pts.append((gy.astype(np.float32) - 128.0, gx.astype(np.float32) - 128.0,
                    py.astype(np.float32) - 128.0, px.astype(np.float32) - 128.0))
        if len(gy) and len(py):
            for q_ix, c_ix in _build_jobs_shared(gy, gx, py, px):
                npc = max(1, -(-len(c_ix) // WP))
                all_jobs.append((npc, i, q_ix, c_ix))

    order = sorted(range(len(all_jobs)),
                   key=lambda k: all_jobs[k][0], reverse=True)
    loads = [0] * N_CORES
    per_core = [[] for _ in range(N_CORES)]
    for k in order:
        c = min(range(N_CORES), key=lambda q: loads[q])
        per_core[c].append(k)
        loads[c] += all_jobs[k][0]

    for c in range(N_CORES):
        per_core[c].sort(key=lambda k: all_jobs[k][0], reverse=True)
    J = max(len(v) for v in per_core)
    slot_w = [0] * J
    for c in range(N_CORES):
        for j, k in enumerate(per_core[c]):
            slot_w[j] = max(slot_w[j], all_jobs[k][0])
    P = sum(slot_w)
    piece_job = np.zeros(P, np.int64)
    slot_off = []
    p = 0
    for j, w in enumerate(slot_w):
        slot_off.append(p)
        piece_job[p:p + w] = j
        p += w

    c0 = min(3 * GT, P)
    stat_split = c0
    rest = P - c0
    bnds = [0, c0] + [c0 + rest * t // 4 for t in (1, 2, 3)] + [P]
    bnds = sorted(set(bnds))
    mov_chunks_sync = [(bnds[t], bnds[t + 1]) for t in range(len(bnds) - 1)]
    mov_chunks_gps = []

    nc = _build_program(P, piece_job, mov_chunks_sync, mov_chunks_gps,
                        stat_split)

    sent_stat = _aug_g(np.empty(0, np.float32), np.empty(0, np.float32), 128)
    sent_mov = _aug_p(np.empty(0, np.float32), np.empty(0, np.float32), WP)
    in_maps = []
    piece_map = []
    for c in range(N_CORES):
        stat = np.empty((6, 128 * P), np.float32)
        mov = np.empty((6, WP * P), np.float32)
        pmap = [None] * P
        for j in range(J):
            p = slot_off[j]
            w = slot_w[j]
            if j < len(per_core[c]):
                k = per_core[c][j]
                npc, i, q_ix, c_ix = all_jobs[k]
                gy, gx, py, px = pts[i]
                sa = _aug_g(gy[q_ix], gx[q_ix], 128)
                aug = _aug_p(py[c_ix], px[c_ix], w * WP)
                for t in range(w):
                    stat[:, (p + t) * 128:(p + t + 1) * 128] = sa
                    mov[:, (p + t) * WP:(p + t + 1) * WP] = \
                        aug[:, t * WP:(t + 1) * WP]
                    pmap[p + t] = k
            else:
                for t in range(w):
                    stat[:, (p + t) * 128:(p + t + 1) * 128] = sent_stat
                    mov[:, (p + t) * WP:(p + t + 1) * WP] = sent_mov
        piece_map.append(pmap)
        in_maps.append({
            "stat": stat.astype(ml_dtypes.bfloat16),
            "mov": mov.astype(ml_dtypes.bfloat16),
        })

    res = run_bass_kernel_spmd(nc, in_maps, list(range(N_CORES)), **RUN_OPTS)
    global LAST_RES, LAST_NN
    LAST_RES = res

    sums = np.zeros((BC, 2), np.float64)
    nn_dbg = {}
    # p->g per-pair NN accumulator over all pred points
    dp_min = [np.full(len(pts[i][2]), np.inf) for i in range(BC)]
    for c in range(N_CORES):
        tmpav = np.asarray(res.results[c]["tmpa"], np.float64)   # [128, P]
        blk = np.asarray(res.results[c]["blk"], np.float64)      # [128, P*WP]
        # group pieces by job slot
        seen = set()
        for pi, k in enumerate(piece_map[c]):
            if k is None or k in seen:
                continue
            seen.add(k)
            npc, i, q_ix, c_ix = all_jobs[k]
            j = per_core[c].index(k)
            p0 = slot_off[j]
            w = slot_w[j]
            nq = len(q_ix)
            # g->p: min over the job's piece columns
            d2 = (tmpav[:nq, p0:p0 + w] * D2_BACK).min(axis=1)
            dist = np.sqrt(np.maximum(d2, 0.0))
            sums[i, 0] += dist.sum()
            nn_dbg.setdefault((i, 0), []).append((q_ix, dist))
            # p->g: per-column max over partitions, scatter-min
            v = blk[:, p0 * WP:p0 * WP + len(c_ix)].max(axis=0)
            d2p = v * D2_BACK
            np.minimum.at(dp_min[i], c_ix, d2p)
    LAST_NN = nn_dbg

    losses = np.full(BC, np.nan, np.float64)
    for i in range(BC):
        gy = pts[i][0]
        py = pts[i][2]
        n_g, n_p = len(gy), len(py)
        if n_g == 0 and n_p == 0:
            continue
        if n_g == 0 or n_p == 0:
            losses[i] = _loss_from_sums(np.inf, max(n_g, 1),
                                        np.inf, max(n_p, 1))
        else:
            sp = np.sqrt(np.maximum(dp_min[i], 0.0)).sum()
            losses[i] = _loss_from_sums(sums[i, 0], n_g, sp, n_p)

    return np.float32(np.nanmean(losses.astype(np.float32)))
